# revision 13
# baseline (speedup 1.0000x reference)
"""Causal self-attention (B=4, T=2048, C=1024, H=16, D=64) on 8 trn2 cores.

Sharding: zero-collective. Core = (batch b, parity p): b = core//2, p = core%2.
Each core handles one batch and 4 interleaved 256-query chunks chosen so the
causal attention work is balanced: parity 0 -> chunks [0,2,5,7], parity 1 ->
[1,3,4,6] (of 8 chunks). Every core computes K/V projections for its full
batch (duplicated across the 2 cores of a batch), attention for its queries,
and the output projection rows for its queries. The SPMD program is identical
across cores; all per-core differences enter through DRAM inputs (xTq slices,
masks, output scatter done on host).

All matmuls run in bf16 (1 cyc/row on the PE at any free size); PSUM
accumulation stays f32. The stages are software-pipelined in one long stream:

  for s in 0..3:  A(2s) A(2s+1) B(s) C(s) D(s)

  A(tch): K^T[:, tch] and V_aug[tch] from one batched x^T chunk DMA.
  B(s):   Q^T for the slot's 256 queries.
  C(s):   flash attention in the key-partition domain: S^T = K^T.T@Q^T ->
          exp (ACT) -> *mask (DVE) -> y^T += V_aug.T @ P^T with a fused
          ones-column giving row sums l.  Normalization entirely on-chip:
          DVE reciprocal of the PSUM l-row, Pool partition_broadcast of 1/l,
          DVE multiply into a resident bf16 y^T tile.
  D(s):   out rows = y^T.T @ Wp^T straight from SBUF, stores via DMA.

Engine split: PE matmuls; ACT exp + K-copies; DVE V/Q-copies, mask, norm;
Pool broadcasts.  All tile loads are single batched DMAs to keep the HWDGE
queue short, ordered so compute starts ~4us in.
"""

import sys

sys.path.insert(0, "/opt/trn_rl_repo")

import numpy as np

import concourse.bass as bass
import concourse.bacc as bacc
import concourse.tile as tile
from concourse import mybir
from concourse.bass_utils import run_bass_kernel_spmd

F32 = mybir.dt.float32
BF16 = mybir.dt.bfloat16

B, T, C, H, D = 4, 2048, 1024, 16, 64
P = 128
NPAIR = H // 2          # 8 head pairs; pair p = heads (2p, 2p+1)
CSUB = C // P           # 8 contraction subtiles
TQL = T // 2            # 1024 local queries per core
NSLOT, QCH = 4, 256     # 4 slots x 256 queries
NJT = T // P            # 16 key tiles of 128
BOUNDS = [4, 8, 12, 16]  # j-tiles processed per slot (uniform across cores)
CHUNKS = [[0, 2, 5, 7], [1, 3, 4, 6]]  # global 256-query chunk per slot
SCALE = 1.0 / 8.0       # 1/sqrt(D)
VW = 132                # V_aug width: [V0(64) | one | one | V1(64)] + pad

_CACHE = {}


def build_nc():
    nc = bacc.Bacc("TRN2", target_bir_lowering=False, debug=False)

    xT = nc.dram_tensor("xT", [C, T], BF16, kind="ExternalInput")
    xTq = nc.dram_tensor("xTq", [C, TQL], BF16, kind="ExternalInput")
    wkT = nc.dram_tensor("wkT", [C, C], BF16, kind="ExternalInput")
    wvT = nc.dram_tensor("wvT", [C, C], BF16, kind="ExternalInput")
    wqT = nc.dram_tensor("wqT", [C, C], BF16, kind="ExternalInput")
    wpT = nc.dram_tensor("wpT", [C, C], BF16, kind="ExternalInput")
    # multiplicative causal mask for the last 4 j-tiles of each slot:
    # [j_local 128, slot, rel_jt 4, q_local 256]
    maskd = nc.dram_tensor("mask", [P, NSLOT, 4, QCH], BF16, kind="ExternalInput")
    out = nc.dram_tensor("out", [TQL, C], F32, kind="ExternalOutput")

    # DRAM views for batched weight/x loads: row (g*128+p) -> [p, g, c]
    wkTv = wkT.rearrange("(g p) c -> p g c", p=P)
    wvTv = wvT.rearrange("(g p) c -> p g c", p=P)
    wqTv = wqT.rearrange("(g p) c -> p g c", p=P)
    wpTv = wpT.rearrange("(g p) c -> p g c", p=P)
    xTv = xT.rearrange("(g p) t -> p g t", p=P)
    xTqv = xTq.rearrange("(g p) t -> p g t", p=P)

    with tile.TileContext(nc) as tc:
        with (
            tc.tile_pool(name="res", bufs=1) as res,
            tc.tile_pool(name="ysp", bufs=2) as ysp,
            tc.tile_pool(name="xp", bufs=2) as xp,
            tc.tile_pool(name="xqp", bufs=2) as xqp,
            tc.tile_pool(name="pmm", bufs=3, space="PSUM") as pmm,
            tc.tile_pool(name="ppy", bufs=2, space="PSUM") as ppy,
            tc.tile_pool(name="wrk", bufs=4) as wrk,
            tc.tile_pool(name="nrm", bufs=3) as nrm,
        )\
        :
            # ---- persistent SBUF residents ----
            wk = res.tile([P, CSUB, C], BF16, name="wk")
            wv = res.tile([P, CSUB, C], BF16, name="wv")
            wq = res.tile([P, CSUB, C], BF16, name="wq")
            wp = res.tile([P, CSUB, C], BF16, name="wp")
            kts = [res.tile([P, NPAIR, QCH], BF16, name=f"kt{i}")
                   for i in range(T // QCH)]           # K^T, 4KB/part each
            vaugs = [res.tile([P, 2, NPAIR, VW], BF16, name=f"va{i}")
                     for i in range(T // QCH)]         # V+ones, ~4.1KB/part each
            qts = [res.tile([P, NPAIR, QCH], BF16, name=f"qt{i}")
                   for i in range(NSLOT)]              # Q^T, 4KB/part each
            mask = res.tile([P, NSLOT, 4, QCH], BF16, name="mask")
            onesb = res.tile([P, 64], BF16, name="onesb")
            nc.vector.memset(onesb, 1.0)

            # ones columns of vaug: col 64 (hi0) and col 129 (hi1)
            for va in vaugs:
                nc.vector.memset(va[:, :, :, 64:65], 1.0)
                nc.vector.memset(va[:, :, :, 129:130], 1.0)

            # warm up the ACT function tables (Exp + Copy) at t~0 so the
            # table DMA overlaps the initial loads
            warm = res.tile([1, 2], F32, name="warm")
            nc.vector.memset(warm, 1.0)
            warm2 = res.tile([1, 2], F32, name="warm2")
            nc.scalar.activation(
                out=warm2, in_=warm,
                func=mybir.ActivationFunctionType.Exp, scale=1.0)
            nc.scalar.copy(out=warm, in_=warm2)

            # ---- initial DMAs, ordered for earliest compute start ----
            nc.sync.dma_start(out=wk[:, 0:4, :], in_=wkTv[:, 0:4, :])
            xts = [None] * (T // QCH)
            xts[0] = xp.tile([P, CSUB, QCH], BF16, tag="xt", name="xt0")
            nc.sync.dma_start(out=xts[0], in_=xTv[:, :, 0:QCH])
            nc.sync.dma_start(out=wk[:, 4:8, :], in_=wkTv[:, 4:8, :])
            nc.sync.dma_start(out=wv[:, 0:4, :], in_=wvTv[:, 0:4, :])
            nc.sync.dma_start(out=wv[:, 4:8, :], in_=wvTv[:, 4:8, :])
            xts[1] = xp.tile([P, CSUB, QCH], BF16, tag="xt", name="xt1")
            nc.sync.dma_start(out=xts[1], in_=xTv[:, :, QCH:2 * QCH])
            nc.sync.dma_start(out=mask, in_=maskd[:, :, :, :])
            nc.sync.dma_start(out=wq[:, 0:4, :], in_=wqTv[:, 0:4, :])
            nc.sync.dma_start(out=wq[:, 4:8, :], in_=wqTv[:, 4:8, :])
            xqs = [None] * NSLOT
            xqs[0] = xqp.tile([P, CSUB, QCH], BF16, tag="xq", name="xq0")
            nc.sync.dma_start(out=xqs[0], in_=xTqv[:, :, 0:QCH])
            nc.sync.dma_start(out=wp[:, 0:4, :], in_=wpTv[:, 0:4, :])
            nc.sync.dma_start(out=wp[:, 4:8, :], in_=wpTv[:, 4:8, :])

            def stage_a(tch):
                """K^T[:, tch] and V_aug[tch] from x^T chunk tch."""
                xt = xts[tch]
                # K: accumulate over cs for 4-pair groups
                for g in range(2):
                    pk = pmm.tile([P, 4, QCH], F32, tag="mm", name="pk")
                    for pp in range(4):
                        p = g * 4 + pp
                        for cs in range(CSUB):
                            nc.tensor.matmul(
                                pk[:, pp, :],
                                wk[:, cs, p * P:(p + 1) * P],
                                xt[:, cs, :],
                                start=(cs == 0), stop=(cs == CSUB - 1),
                            )
                    nc.scalar.copy(
                        out=kts[tch][:, g * 4:(g + 1) * 4, :], in_=pk)
                # V: two 128-row subtiles per chunk, two 512-col halves
                for ts in range(2):
                    for och in range(2):
                        pv = pmm.tile([P, 4, QCH], F32, tag="mm", name="pv")
                        pvv = pv.rearrange("a b c -> a (b c)")[:, 0:512]
                        for cs in range(CSUB):
                            nc.tensor.matmul(
                                pvv,
                                xt[:, cs, ts * P:(ts + 1) * P],
                                wv[:, cs, och * 512:(och + 1) * 512],
                                start=(cs == 0), stop=(cs == CSUB - 1),
                            )
                        # pv cols = (pair-in-half 4, hi 2, d 64)
                        pvh = pvv.rearrange("a (b s d) -> a b s d", b=4, s=2)
                        p4 = slice(och * 4, (och + 1) * 4)
                        nc.vector.tensor_copy(
                            out=vaugs[tch][:, ts, p4, 0:64],
                            in_=pvh[:, :, 0, :])
                        nc.vector.tensor_copy(
                            out=vaugs[tch][:, ts, p4, 65:129],
                            in_=pvh[:, :, 1, :])

            def stage_b(s):
                """Q^T for slot s."""
                xq = xqs[s]
                for g in range(2):
                    pq = pmm.tile([P, 4, QCH], F32, tag="mm", name="pq")
                    for pp in range(4):
                        p = g * 4 + pp
                        for cs in range(CSUB):
                            nc.tensor.matmul(
                                pq[:, pp, :],
                                wq[:, cs, p * P:(p + 1) * P],
                                xq[:, cs, :],
                                start=(cs == 0), stop=(cs == CSUB - 1),
                            )
                    nc.vector.tensor_copy(
                        out=qts[s][:, g * 4:(g + 1) * 4, :], in_=pq)

            def stage_c(s, ysT):
                """Attention for slot s into resident bf16 y^T tile."""
                nj = BOUNDS[s]
                ngrp = nj // 4
                for p in range(NPAIR):
                    ypp = ppy.tile([P, 2, QCH], F32, tag="ypp", name="ypp")
                    for hi in range(2):
                        h0 = hi * 64
                        # both heads: rows 0..63 y, row 64 l
                        yout = ypp[0:65, hi, :]
                        vsl = slice(hi * 65, hi * 65 + 65)  # [V | one]
                        for g in range(ngrp):
                            st4 = pmm.tile([P, 4, QCH], F32, tag="mm", name="st4")
                            for i in range(4):
                                jt = g * 4 + i
                                nc.tensor.matmul(
                                    st4[:, i, :],
                                    kts[jt // 2][h0:h0 + 64, p,
                                                 (jt % 2) * P:(jt % 2 + 1) * P],
                                    qts[s][h0:h0 + 64, p, :],
                                    start=True, stop=True,
                                )
                            pt4 = wrk.tile([P, 4, QCH], BF16, tag="pt", name="pt4")
                            nc.scalar.activation(
                                out=pt4, in_=st4,
                                func=mybir.ActivationFunctionType.Exp,
                                scale=SCALE,
                            )
                            if g == ngrp - 1:
                                nc.vector.tensor_mul(pt4, pt4, mask[:, s, :, :])
                            for i in range(4):
                                jt = g * 4 + i
                                nc.tensor.matmul(
                                    yout,
                                    vaugs[jt // 2][:, jt % 2, p, vsl],
                                    pt4[:, i, :],
                                    start=(jt == 0), stop=(jt == nj - 1),
                                )
                    # normalization, fully on-chip: 1/l on DVE, broadcast of
                    # 1/l across partitions via a rank-1 PE matmul
                    for hi in range(2):
                        rinv = nrm.tile([P, QCH], BF16, tag="rinv", name="rinv")
                        with nc.allow_low_precision(reason="1/l in bf16"):
                            nc.vector.reciprocal(
                                out=rinv[64:65, :], in_=ypp[64:65, hi, :])
                        lb = pmm.tile([P, 4, QCH], F32, tag="mm", name="lb")
                        nc.tensor.matmul(
                            lb[0:64, 0, :], onesb[64:65, :], rinv[64:65, :],
                            start=True, stop=True,
                        )
                        # DVE reads only one PSUM operand: copy y to SBUF
                        # first, then scale by the PSUM broadcast tile
                        yc = nrm.tile([64, QCH], BF16, tag="yc", name="yc")
                        nc.vector.tensor_copy(out=yc, in_=ypp[0:64, hi, :])
                        if hi == 0:
                            nc.vector.tensor_mul(
                                ysT[0:64, p, :], yc, lb[0:64, 0, :])
                        else:
                            # hi1 lives at ysT partitions 64..127: go through
                            # a base-0 staging tile, then a lane-crossing
                            # SBUF->SBUF DMA
                            ysb = nrm.tile([64, QCH], BF16, tag="ysb",
                                           name="ysb")
                            nc.vector.tensor_mul(
                                ysb, yc, lb[0:64, 0, :])
                            nc.sync.dma_start(
                                out=ysT[64:128, p, :], in_=ysb)

            def stage_d(s, ysT):
                """Output projection for slot s from resident y^T."""
                for qh in range(2):
                    for och in range(2):
                        po = pmm.tile([P, 4, QCH], F32, tag="mm", name="po")
                        pov = po.rearrange("a b c -> a (b c)")[:, 0:512]
                        for cb in range(CSUB):
                            nc.tensor.matmul(
                                pov,
                                ysT[:, cb, qh * P:(qh + 1) * P],
                                wp[:, cb, och * 512:(och + 1) * 512],
                                start=(cb == 0), stop=(cb == CSUB - 1),
                            )
                        osb = wrk.tile([P, 512], F32, tag="osb", name="osb")
                        nc.vector.tensor_copy(out=osb, in_=pov)
                        nc.sync.dma_start(
                            out=out[s * QCH + qh * P:s * QCH + (qh + 1) * P,
                                    och * 512:(och + 1) * 512],
                            in_=osb,
                        )

            # ---- the pipeline ----
            for s in range(NSLOT):
                for half in range(2):
                    tch = 2 * s + half
                    if xts[tch] is None:
                        xts[tch] = xp.tile(
                            [P, CSUB, QCH], BF16, tag="xt", name=f"xt{tch}")
                        nc.sync.dma_start(
                            out=xts[tch],
                            in_=xTv[:, :, tch * QCH:(tch + 1) * QCH])
                    stage_a(tch)
                if xqs[s] is None:
                    xqs[s] = xqp.tile(
                        [P, CSUB, QCH], BF16, tag="xq", name=f"xq{s}")
                    nc.sync.dma_start(
                        out=xqs[s],
                        in_=xTqv[:, :, s * QCH:(s + 1) * QCH])
                stage_b(s)
                ysT = ysp.tile([P, NPAIR, QCH], BF16, tag="ys", name=f"ys{s}")
                stage_c(s, ysT)
                stage_d(s, ysT)

    nc.compile()
    return nc


def _make_mask(parity: int) -> np.ndarray:
    import ml_dtypes
    m = np.zeros((P, NSLOT, 4, QCH), dtype=np.float32)
    for s in range(NSLOT):
        c = CHUNKS[parity][s]
        for i in range(4):
            jt = BOUNDS[s] - 4 + i
            jg = jt * P + np.arange(P)[:, None]          # key index
            qg = c * QCH + np.arange(QCH)[None, :]       # query index
            m[:, s, i, :] = np.where(jg <= qg, 1.0, 0.0)
    return m.astype(ml_dtypes.bfloat16)


def kernel(x, Wq, bq, Wk, bk, Wv, bv, Wp, bp):
    import ml_dtypes
    x = np.asarray(x, dtype=np.float32)
    assert x.shape == (B, T, C)
    for b_ in (bq, bk, bv, bp):
        assert not np.any(np.asarray(b_)), "nonzero biases unsupported"

    if "nc" not in _CACHE:
        _CACHE["nc"] = build_nc()
    nc = _CACHE["nc"]

    bf = ml_dtypes.bfloat16
    wqT = np.ascontiguousarray(np.asarray(Wq, np.float32).T).astype(bf)
    wkT = np.ascontiguousarray(np.asarray(Wk, np.float32).T).astype(bf)
    wvT = np.ascontiguousarray(np.asarray(Wv, np.float32).T).astype(bf)
    wpT = np.ascontiguousarray(np.asarray(Wp, np.float32).T).astype(bf)
    masks = [_make_mask(0), _make_mask(1)]

    in_maps = []
    for core in range(8):
        b, par = core // 2, core % 2
        xTf = np.ascontiguousarray(x[b].T)
        xT = xTf.astype(bf)
        xTq = np.ascontiguousarray(
            np.concatenate(
                [xTf[:, c * QCH:(c + 1) * QCH] for c in CHUNKS[par]], axis=1)
        ).astype(bf)
        in_maps.append(
            dict(xT=xT, xTq=xTq, wqT=wqT, wkT=wkT, wvT=wvT, wpT=wpT,
                 mask=masks[par])
        )

    _CACHE["last_in_maps"] = in_maps
    try:
        res = run_bass_kernel_spmd(nc, in_maps, core_ids=list(range(8)))
    except Exception:
        # the axon device occasionally reports NRT_EXEC_UNIT_UNRECOVERABLE;
        # resetting the PJRT backend and retrying once recovers it
        import jax
        try:
            jax.clear_caches()
            jax.extend.backend.clear_backends()
        except Exception:
            pass
        res = run_bass_kernel_spmd(nc, in_maps, core_ids=list(range(8)))

    out = np.empty((B, T, C), dtype=np.float32)
    for core in range(8):
        ol = res.results[core]["out"]
        b, par = core // 2, core % 2
        for s, c in enumerate(CHUNKS[par]):
            out[b, c * QCH:(c + 1) * QCH] = ol[s * QCH:(s + 1) * QCH]
    return out


# revision 15
# speedup vs baseline: 1.2990x; 1.2990x over previous
"""Causal self-attention (B=4, T=2048, C=1024, H=16, D=64) on 8 trn2 cores.

Sharding: zero-collective. Core = (batch b, parity p): b = core//2, p = core%2.
Each core handles one batch and 4 interleaved 256-query chunks chosen so the
causal attention work is balanced: parity 0 -> chunks [0,2,5,7], parity 1 ->
[1,3,4,6] (of 8 chunks). Every core computes K/V projections for its full
batch (duplicated across the 2 cores of a batch), attention for its queries,
and the output projection rows for its queries. The SPMD program is identical
across cores; all per-core differences enter through DRAM inputs (xTq slices,
masks, output scatter done on host).

All matmuls run in bf16 (1 cyc/row on the PE at any free size); PSUM
accumulation stays f32. The stages are software-pipelined in one long stream:

  for s in 0..3:  A(2s) A(2s+1) B(s) C(s) D(s)

  A(tch): K^T[:, tch] and V_aug[tch] from one batched x^T chunk DMA.
  B(s):   Q^T for the slot's 256 queries.
  C(s):   flash attention in the key-partition domain: S^T = K^T.T@Q^T ->
          exp (ACT) -> *mask (DVE) -> y^T += V_aug.T @ P^T with a fused
          ones-column giving row sums l.  Normalization entirely on-chip:
          DVE reciprocal of the PSUM l-row, Pool partition_broadcast of 1/l,
          DVE multiply into a resident bf16 y^T tile.
  D(s):   out rows = y^T.T @ Wp^T straight from SBUF, stores via DMA.

Engine split: PE matmuls; ACT exp + K-copies; DVE V/Q-copies, mask, norm;
Pool broadcasts.  All tile loads are single batched DMAs to keep the HWDGE
queue short, ordered so compute starts ~4us in.
"""

import sys

sys.path.insert(0, "/opt/trn_rl_repo")

import numpy as np

import concourse.bass as bass
import concourse.bacc as bacc
import concourse.tile as tile
from concourse import mybir
from concourse.bass_utils import run_bass_kernel_spmd

F32 = mybir.dt.float32
BF16 = mybir.dt.bfloat16

B, T, C, H, D = 4, 2048, 1024, 16, 64
P = 128
NPAIR = H // 2          # 8 head pairs; pair p = heads (2p, 2p+1)
CSUB = C // P           # 8 contraction subtiles
TQL = T // 2            # 1024 local queries per core
NSLOT, QCH = 4, 256     # 4 slots x 256 queries
NJT = T // P            # 16 key tiles of 128
BOUNDS = [4, 8, 12, 16]  # j-tiles processed per slot (uniform across cores)
CHUNKS = [[0, 2, 5, 7], [1, 3, 4, 6]]  # global 256-query chunk per slot
SCALE = 1.0 / 8.0       # 1/sqrt(D)
VW = 132                # V_aug width: [V0(64) | one | one | V1(64)] + pad

_CACHE = {}


def build_nc():
    nc = bacc.Bacc("TRN2", target_bir_lowering=False, debug=False)

    xT = nc.dram_tensor("xT", [C, T], BF16, kind="ExternalInput")
    xTq = nc.dram_tensor("xTq", [C, TQL], BF16, kind="ExternalInput")
    wkT = nc.dram_tensor("wkT", [C, C], BF16, kind="ExternalInput")
    wvT = nc.dram_tensor("wvT", [C, C], BF16, kind="ExternalInput")
    wqT = nc.dram_tensor("wqT", [C, C], BF16, kind="ExternalInput")
    wpT = nc.dram_tensor("wpT", [C, C], BF16, kind="ExternalInput")
    # multiplicative causal mask for the last 4 j-tiles of each slot:
    # [j_local 128, slot, rel_jt 4, q_local 256]
    maskd = nc.dram_tensor("mask", [P, NSLOT, 4, QCH], BF16, kind="ExternalInput")
    out = nc.dram_tensor("out", [TQL, C], F32, kind="ExternalOutput")

    # DRAM views for batched weight/x loads: row (g*128+p) -> [p, g, c]
    wkTv = wkT.rearrange("(g p) c -> p g c", p=P)
    wvTv = wvT.rearrange("(g p) c -> p g c", p=P)
    wqTv = wqT.rearrange("(g p) c -> p g c", p=P)
    wpTv = wpT.rearrange("(g p) c -> p g c", p=P)
    xTv = xT.rearrange("(g p) t -> p g t", p=P)
    xTqv = xTq.rearrange("(g p) t -> p g t", p=P)

    with tile.TileContext(nc) as tc:
        with (
            tc.tile_pool(name="res", bufs=1) as res,
            tc.tile_pool(name="ysp", bufs=2) as ysp,
            tc.tile_pool(name="xp", bufs=2) as xp,
            tc.tile_pool(name="xqp", bufs=2) as xqp,
            tc.tile_pool(name="pmm", bufs=3, space="PSUM") as pmm,
            tc.tile_pool(name="ppy", bufs=2, space="PSUM") as ppy,
            tc.tile_pool(name="wrk", bufs=4) as wrk,
            tc.tile_pool(name="nrm", bufs=3) as nrm,
        )\
        :
            # ---- persistent SBUF residents ----
            wk = res.tile([P, CSUB, C], BF16, name="wk")
            wv = res.tile([P, CSUB, C], BF16, name="wv")
            wq = res.tile([P, CSUB, C], BF16, name="wq")
            wp = res.tile([P, CSUB, C], BF16, name="wp")
            kts = [res.tile([P, NPAIR, QCH], BF16, name=f"kt{i}")
                   for i in range(T // QCH)]           # K^T, 4KB/part each
            vaugs = [res.tile([P, 2, NPAIR, VW], BF16, name=f"va{i}")
                     for i in range(T // QCH)]         # V+ones, ~4.1KB/part each
            qts = [res.tile([P, NPAIR, QCH], BF16, name=f"qt{i}")
                   for i in range(NSLOT)]              # Q^T, 4KB/part each
            mask = res.tile([P, NSLOT, 4, QCH], BF16, name="mask")
            onesb = res.tile([P, 64], BF16, name="onesb")
            nc.vector.memset(onesb, 1.0)

            # ones columns of vaug: col 64 (hi0) and col 129 (hi1)
            for va in vaugs:
                nc.vector.memset(va[:, :, :, 64:65], 1.0)
                nc.vector.memset(va[:, :, :, 129:130], 1.0)

            # warm up the ACT function tables (Exp + Copy) at t~0 so the
            # table DMA overlaps the initial loads
            warm = res.tile([1, 2], F32, name="warm")
            nc.vector.memset(warm, 1.0)
            warm2 = res.tile([1, 2], F32, name="warm2")
            nc.scalar.activation(
                out=warm2, in_=warm,
                func=mybir.ActivationFunctionType.Exp, scale=1.0)
            nc.scalar.copy(out=warm, in_=warm2)

            # ---- initial DMAs, ordered for earliest compute start ----
            nc.sync.dma_start(out=wk[:, 0:4, :], in_=wkTv[:, 0:4, :])
            xts = [None] * (T // QCH)
            xts[0] = xp.tile([P, CSUB, QCH], BF16, tag="xt", name="xt0")
            nc.sync.dma_start(out=xts[0], in_=xTv[:, :, 0:QCH])
            nc.sync.dma_start(out=wk[:, 4:8, :], in_=wkTv[:, 4:8, :])
            nc.sync.dma_start(out=wv[:, 0:4, :], in_=wvTv[:, 0:4, :])
            nc.sync.dma_start(out=wv[:, 4:8, :], in_=wvTv[:, 4:8, :])
            xts[1] = xp.tile([P, CSUB, QCH], BF16, tag="xt", name="xt1")
            nc.sync.dma_start(out=xts[1], in_=xTv[:, :, QCH:2 * QCH])
            nc.sync.dma_start(out=mask, in_=maskd[:, :, :, :])
            nc.sync.dma_start(out=wq[:, 0:4, :], in_=wqTv[:, 0:4, :])
            nc.sync.dma_start(out=wq[:, 4:8, :], in_=wqTv[:, 4:8, :])
            xqs = [None] * NSLOT
            xqs[0] = xqp.tile([P, CSUB, QCH], BF16, tag="xq", name="xq0")
            nc.sync.dma_start(out=xqs[0], in_=xTqv[:, :, 0:QCH])
            nc.sync.dma_start(out=wp[:, 0:4, :], in_=wpTv[:, 0:4, :])
            nc.sync.dma_start(out=wp[:, 4:8, :], in_=wpTv[:, 4:8, :])

            def stage_a(tch):
                """K^T[:, tch] and V_aug[tch] from x^T chunk tch."""
                xt = xts[tch]
                # K: accumulate over cs for 4-pair groups
                for g in range(2):
                    pk = pmm.tile([P, 4, QCH], F32, tag="mm", name="pk")
                    for pp in range(4):
                        p = g * 4 + pp
                        for cs in range(CSUB):
                            nc.tensor.matmul(
                                pk[:, pp, :],
                                wk[:, cs, p * P:(p + 1) * P],
                                xt[:, cs, :],
                                start=(cs == 0), stop=(cs == CSUB - 1),
                            )
                    nc.scalar.copy(
                        out=kts[tch][:, g * 4:(g + 1) * 4, :], in_=pk)
                # V: two 128-row subtiles per chunk, two 512-col halves
                for ts in range(2):
                    for och in range(2):
                        pv = pmm.tile([P, 4, QCH], F32, tag="mm", name="pv")
                        pvv = pv.rearrange("a b c -> a (b c)")[:, 0:512]
                        for cs in range(CSUB):
                            nc.tensor.matmul(
                                pvv,
                                xt[:, cs, ts * P:(ts + 1) * P],
                                wv[:, cs, och * 512:(och + 1) * 512],
                                start=(cs == 0), stop=(cs == CSUB - 1),
                            )
                        # pv cols = (pair-in-half 4, hi 2, d 64)
                        pvh = pvv.rearrange("a (b s d) -> a b s d", b=4, s=2)
                        p4 = slice(och * 4, (och + 1) * 4)
                        nc.vector.tensor_copy(
                            out=vaugs[tch][:, ts, p4, 0:64],
                            in_=pvh[:, :, 0, :])
                        nc.vector.tensor_copy(
                            out=vaugs[tch][:, ts, p4, 65:129],
                            in_=pvh[:, :, 1, :])

            def stage_b(s):
                """Q^T for slot s."""
                xq = xqs[s]
                for g in range(2):
                    pq = pmm.tile([P, 4, QCH], F32, tag="mm", name="pq")
                    for pp in range(4):
                        p = g * 4 + pp
                        for cs in range(CSUB):
                            nc.tensor.matmul(
                                pq[:, pp, :],
                                wq[:, cs, p * P:(p + 1) * P],
                                xq[:, cs, :],
                                start=(cs == 0), stop=(cs == CSUB - 1),
                            )
                    nc.vector.tensor_copy(
                        out=qts[s][:, g * 4:(g + 1) * 4, :], in_=pq)

            def stage_c(s, ysT):
                """Attention for slot s into resident bf16 y^T tile."""
                nj = BOUNDS[s]
                ngrp = nj // 4

                def norm(p, ypp):
                    # batched 1/l for both heads, rank-1 PE broadcast, then
                    # one SBUF copy of the broadcast + two scale-muls
                    rinv = nrm.tile([P, 2, QCH], BF16, tag="rinv", name="rinv")
                    with nc.allow_low_precision(reason="1/l in bf16"):
                        nc.vector.reciprocal(
                            out=rinv[64:65, :, :], in_=ypp[64:65, :, :])
                    lb = pmm.tile([P, 4, QCH], F32, tag="mm", name="lb")
                    nc.tensor.matmul(
                        lb[0:64, 0:2, :], onesb[64:65, :], rinv[64:65, :, :],
                        start=True, stop=True,
                    )
                    lbs = nrm.tile([64, 2, QCH], BF16, tag="lbs", name="lbs")
                    nc.vector.tensor_copy(out=lbs, in_=lb[0:64, 0:2, :])
                    nc.vector.tensor_mul(
                        ysT[0:64, p, :], ypp[0:64, 0, :], lbs[:, 0, :])
                    # hi1 lives at ysT partitions 64..127: go through a
                    # base-0 staging tile + lane-crossing SBUF->SBUF DMA
                    ysb = nrm.tile([64, QCH], BF16, tag="ysb", name="ysb")
                    nc.vector.tensor_mul(
                        ysb, ypp[0:64, 1, :], lbs[:, 1, :])
                    nc.sync.dma_start(out=ysT[64:128, p, :], in_=ysb)

                ypps = [None] * NPAIR
                for p in range(NPAIR):
                    ypp = ppy.tile([P, 2, QCH], F32, tag="ypp", name="ypp")
                    ypps[p] = ypp
                    for hi in range(2):
                        h0 = hi * 64
                        # both heads: rows 0..63 y, row 64 l
                        yout = ypp[0:65, hi, :]
                        vsl = slice(hi * 65, hi * 65 + 65)  # [V | one]
                        for g in range(ngrp):
                            st4 = pmm.tile([P, 4, QCH], F32, tag="mm", name="st4")
                            for i in range(4):
                                jt = g * 4 + i
                                nc.tensor.matmul(
                                    st4[:, i, :],
                                    kts[jt // 2][h0:h0 + 64, p,
                                                 (jt % 2) * P:(jt % 2 + 1) * P],
                                    qts[s][h0:h0 + 64, p, :],
                                    start=True, stop=True,
                                )
                            pt4 = wrk.tile([P, 4, QCH], BF16, tag="pt", name="pt4")
                            nc.scalar.activation(
                                out=pt4, in_=st4,
                                func=mybir.ActivationFunctionType.Exp,
                                scale=SCALE,
                            )
                            if g == ngrp - 1:
                                nc.vector.tensor_mul(pt4, pt4, mask[:, s, :, :])
                            for i in range(4):
                                jt = g * 4 + i
                                nc.tensor.matmul(
                                    yout,
                                    vaugs[jt // 2][:, jt % 2, p, vsl],
                                    pt4[:, i, :],
                                    start=(jt == 0), stop=(jt == nj - 1),
                                )
                    # normalization of the previous pair (deferred so the
                    # norm's PE broadcast never stalls this pair's matmuls)
                    if p >= 1:
                        norm(p - 1, ypps[p - 1])
                norm(NPAIR - 1, ypps[NPAIR - 1])

            def stage_d(s, ysT):
                """Output projection for slot s from resident y^T."""
                for qh in range(2):
                    for och in range(2):
                        po = pmm.tile([P, 4, QCH], F32, tag="mm", name="po")
                        pov = po.rearrange("a b c -> a (b c)")[:, 0:512]
                        for cb in range(CSUB):
                            nc.tensor.matmul(
                                pov,
                                ysT[:, cb, qh * P:(qh + 1) * P],
                                wp[:, cb, och * 512:(och + 1) * 512],
                                start=(cb == 0), stop=(cb == CSUB - 1),
                            )
                        osb = wrk.tile([P, 512], F32, tag="osb", name="osb")
                        nc.vector.tensor_copy(out=osb, in_=pov)
                        nc.sync.dma_start(
                            out=out[s * QCH + qh * P:s * QCH + (qh + 1) * P,
                                    och * 512:(och + 1) * 512],
                            in_=osb,
                        )

            # ---- the pipeline ----
            for s in range(NSLOT):
                for half in range(2):
                    tch = 2 * s + half
                    if xts[tch] is None:
                        xts[tch] = xp.tile(
                            [P, CSUB, QCH], BF16, tag="xt", name=f"xt{tch}")
                        nc.sync.dma_start(
                            out=xts[tch],
                            in_=xTv[:, :, tch * QCH:(tch + 1) * QCH])
                    stage_a(tch)
                if xqs[s] is None:
                    xqs[s] = xqp.tile(
                        [P, CSUB, QCH], BF16, tag="xq", name=f"xq{s}")
                    nc.sync.dma_start(
                        out=xqs[s],
                        in_=xTqv[:, :, s * QCH:(s + 1) * QCH])
                stage_b(s)
                ysT = ysp.tile([P, NPAIR, QCH], BF16, tag="ys", name=f"ys{s}")
                stage_c(s, ysT)
                stage_d(s, ysT)

    nc.compile()
    return nc


def _make_mask(parity: int) -> np.ndarray:
    import ml_dtypes
    m = np.zeros((P, NSLOT, 4, QCH), dtype=np.float32)
    for s in range(NSLOT):
        c = CHUNKS[parity][s]
        for i in range(4):
            jt = BOUNDS[s] - 4 + i
            jg = jt * P + np.arange(P)[:, None]          # key index
            qg = c * QCH + np.arange(QCH)[None, :]       # query index
            m[:, s, i, :] = np.where(jg <= qg, 1.0, 0.0)
    return m.astype(ml_dtypes.bfloat16)


def kernel(x, Wq, bq, Wk, bk, Wv, bv, Wp, bp):
    import ml_dtypes
    x = np.asarray(x, dtype=np.float32)
    assert x.shape == (B, T, C)
    for b_ in (bq, bk, bv, bp):
        assert not np.any(np.asarray(b_)), "nonzero biases unsupported"

    if "nc" not in _CACHE:
        _CACHE["nc"] = build_nc()
    nc = _CACHE["nc"]

    bf = ml_dtypes.bfloat16
    wqT = np.ascontiguousarray(np.asarray(Wq, np.float32).T).astype(bf)
    wkT = np.ascontiguousarray(np.asarray(Wk, np.float32).T).astype(bf)
    wvT = np.ascontiguousarray(np.asarray(Wv, np.float32).T).astype(bf)
    wpT = np.ascontiguousarray(np.asarray(Wp, np.float32).T).astype(bf)
    masks = [_make_mask(0), _make_mask(1)]

    in_maps = []
    for core in range(8):
        b, par = core // 2, core % 2
        xTf = np.ascontiguousarray(x[b].T)
        xT = xTf.astype(bf)
        xTq = np.ascontiguousarray(
            np.concatenate(
                [xTf[:, c * QCH:(c + 1) * QCH] for c in CHUNKS[par]], axis=1)
        ).astype(bf)
        in_maps.append(
            dict(xT=xT, xTq=xTq, wqT=wqT, wkT=wkT, wvT=wvT, wpT=wpT,
                 mask=masks[par])
        )

    _CACHE["last_in_maps"] = in_maps
    try:
        res = run_bass_kernel_spmd(nc, in_maps, core_ids=list(range(8)))
    except Exception:
        # the axon device occasionally reports NRT_EXEC_UNIT_UNRECOVERABLE;
        # resetting the PJRT backend and retrying once recovers it
        import jax
        try:
            jax.clear_caches()
            jax.extend.backend.clear_backends()
        except Exception:
            pass
        res = run_bass_kernel_spmd(nc, in_maps, core_ids=list(range(8)))

    out = np.empty((B, T, C), dtype=np.float32)
    for core in range(8):
        ol = res.results[core]["out"]
        b, par = core // 2, core % 2
        for s, c in enumerate(CHUNKS[par]):
            out[b, c * QCH:(c + 1) * QCH] = ol[s * QCH:(s + 1) * QCH]
    return out


# revision 20
# speedup vs baseline: 1.3077x; 1.0067x over previous
"""Causal self-attention (B=4, T=2048, C=1024, H=16, D=64) on 8 trn2 cores.

Sharding: zero-collective. Core = (batch b, parity p): b = core//2, p = core%2.
Each core handles one batch and 4 interleaved 256-query chunks chosen so the
causal attention work is balanced: parity 0 -> chunks [0,2,5,7], parity 1 ->
[1,3,4,6] (of 8 chunks). Every core computes K/V projections for its full
batch (duplicated across the 2 cores of a batch), attention for its queries,
and the output projection rows for its queries. The SPMD program is identical
across cores; all per-core differences enter through DRAM inputs (xTq slices,
masks, output scatter done on host).

All matmuls run in bf16 (1 cyc/row on the PE at any free size); PSUM
accumulation stays f32. The stages are software-pipelined in one long stream:

  for s in 0..3:  A(2s) A(2s+1) B(s) C(s) D(s)

  A(tch): K^T[:, tch] and V_aug[tch] from one batched x^T chunk DMA.
  B(s):   Q^T for the slot's 256 queries.
  C(s):   flash attention in the key-partition domain: S^T = K^T.T@Q^T ->
          exp (ACT) -> *mask (DVE) -> y^T += V_aug.T @ P^T with a fused
          ones-column giving row sums l.  Normalization entirely on-chip:
          DVE reciprocal of the PSUM l-row, Pool partition_broadcast of 1/l,
          DVE multiply into a resident bf16 y^T tile.
  D(s):   out rows = y^T.T @ Wp^T straight from SBUF, stores via DMA.

Engine split: PE matmuls; ACT exp + K-copies; DVE V/Q-copies, mask, norm;
Pool broadcasts.  All tile loads are single batched DMAs to keep the HWDGE
queue short, ordered so compute starts ~4us in.
"""

import sys

sys.path.insert(0, "/opt/trn_rl_repo")

import numpy as np

import concourse.bass as bass
import concourse.bacc as bacc
import concourse.tile as tile
from concourse import mybir
from concourse.bass_utils import run_bass_kernel_spmd

F32 = mybir.dt.float32
BF16 = mybir.dt.bfloat16

B, T, C, H, D = 4, 2048, 1024, 16, 64
P = 128
NPAIR = H // 2          # 8 head pairs; pair p = heads (2p, 2p+1)
CSUB = C // P           # 8 contraction subtiles
TQL = T // 2            # 1024 local queries per core
NSLOT, QCH = 4, 256     # 4 slots x 256 queries
NJT = T // P            # 16 key tiles of 128
BOUNDS = [4, 8, 12, 16]  # j-tiles processed per slot (uniform across cores)
CHUNKS = [[0, 2, 5, 7], [1, 3, 4, 6]]  # global 256-query chunk per slot
SCALE = 1.0 / 8.0       # 1/sqrt(D)
VW = 132                # V_aug width: [V0(64) | one | one | V1(64)] + pad

_CACHE = {}


def build_nc():
    nc = bacc.Bacc("TRN2", target_bir_lowering=False, debug=False)

    xT = nc.dram_tensor("xT", [C, T], BF16, kind="ExternalInput")
    xTq = nc.dram_tensor("xTq", [C, TQL], BF16, kind="ExternalInput")
    wkT = nc.dram_tensor("wkT", [C, C], BF16, kind="ExternalInput")
    wvT = nc.dram_tensor("wvT", [C, C], BF16, kind="ExternalInput")
    wqT = nc.dram_tensor("wqT", [C, C], BF16, kind="ExternalInput")
    wpT = nc.dram_tensor("wpT", [C, C], BF16, kind="ExternalInput")
    # multiplicative causal mask for the last 4 j-tiles of each slot:
    # [j_local 128, slot, rel_jt 4, q_local 256]
    maskd = nc.dram_tensor("mask", [P, NSLOT, 4, QCH], BF16, kind="ExternalInput")
    out = nc.dram_tensor("out", [TQL, C], F32, kind="ExternalOutput")

    # DRAM views for batched weight/x loads: row (g*128+p) -> [p, g, c]
    wkTv = wkT.rearrange("(g p) c -> p g c", p=P)
    wvTv = wvT.rearrange("(g p) c -> p g c", p=P)
    wqTv = wqT.rearrange("(g p) c -> p g c", p=P)
    wpTv = wpT.rearrange("(g p) c -> p g c", p=P)
    xTv = xT.rearrange("(g p) t -> p g t", p=P)
    xTqv = xTq.rearrange("(g p) t -> p g t", p=P)

    with tile.TileContext(nc) as tc:
        with (
            tc.tile_pool(name="res", bufs=1) as res,
            tc.tile_pool(name="ysp", bufs=2) as ysp,
            tc.tile_pool(name="xp", bufs=2) as xp,
            tc.tile_pool(name="xqp", bufs=2) as xqp,
            tc.tile_pool(name="pmm", bufs=3, space="PSUM") as pmm,
            tc.tile_pool(name="ppy", bufs=2, space="PSUM") as ppy,
            tc.tile_pool(name="wrk", bufs=4) as wrk,
            tc.tile_pool(name="nrm", bufs=3) as nrm,
        )\
        :
            # ---- persistent SBUF residents ----
            wk = res.tile([P, CSUB, C], BF16, name="wk")
            wv = res.tile([P, CSUB, C], BF16, name="wv")
            wq = res.tile([P, CSUB, C], BF16, name="wq")
            wp = res.tile([P, CSUB, C], BF16, name="wp")
            kts = [res.tile([P, NPAIR, QCH], BF16, name=f"kt{i}")
                   for i in range(T // QCH)]           # K^T, 4KB/part each
            vaugs = [res.tile([P, 2, NPAIR, VW], BF16, name=f"va{i}")
                     for i in range(T // QCH)]         # V+ones, ~4.1KB/part each
            qts = [res.tile([P, NPAIR, QCH], BF16, name=f"qt{i}")
                   for i in range(NSLOT)]              # Q^T, 4KB/part each
            mask = res.tile([P, NSLOT, 4, QCH], BF16, name="mask")
            onesb = res.tile([P, 64], BF16, name="onesb")
            nc.vector.memset(onesb, 1.0)

            # ones columns of vaug: col 64 (hi0) and col 129 (hi1)
            for va in vaugs:
                nc.vector.memset(va[:, :, :, 64:65], 1.0)
                nc.vector.memset(va[:, :, :, 129:130], 1.0)

            # warm up the ACT function tables (Exp + Copy) at t~0 so the
            # table DMA overlaps the initial loads
            warm = res.tile([1, 2], F32, name="warm")
            nc.vector.memset(warm, 1.0)
            warm2 = res.tile([1, 2], F32, name="warm2")
            nc.scalar.activation(
                out=warm2, in_=warm,
                func=mybir.ActivationFunctionType.Exp, scale=1.0)
            nc.scalar.copy(out=warm, in_=warm2)

            # ---- initial DMAs, ordered for earliest compute start ----
            nc.sync.dma_start(out=wk[:, 0:4, :], in_=wkTv[:, 0:4, :])
            xts = [None] * (T // QCH)
            xts[0] = xp.tile([P, CSUB, QCH], BF16, tag="xt", name="xt0")
            nc.sync.dma_start(out=xts[0], in_=xTv[:, :, 0:QCH])
            nc.sync.dma_start(out=wk[:, 4:8, :], in_=wkTv[:, 4:8, :])
            nc.sync.dma_start(out=wv[:, 0:4, :], in_=wvTv[:, 0:4, :])
            nc.sync.dma_start(out=wv[:, 4:8, :], in_=wvTv[:, 4:8, :])
            xts[1] = xp.tile([P, CSUB, QCH], BF16, tag="xt", name="xt1")
            nc.sync.dma_start(out=xts[1], in_=xTv[:, :, QCH:2 * QCH])
            nc.sync.dma_start(out=mask, in_=maskd[:, :, :, :])
            nc.sync.dma_start(out=wq[:, 0:4, :], in_=wqTv[:, 0:4, :])
            nc.sync.dma_start(out=wq[:, 4:8, :], in_=wqTv[:, 4:8, :])
            xqs = [None] * NSLOT
            xqs[0] = xqp.tile([P, CSUB, QCH], BF16, tag="xq", name="xq0")
            nc.sync.dma_start(out=xqs[0], in_=xTqv[:, :, 0:QCH])
            nc.sync.dma_start(out=wp[:, 0:4, :], in_=wpTv[:, 0:4, :])
            nc.sync.dma_start(out=wp[:, 4:8, :], in_=wpTv[:, 4:8, :])

            def stage_a(tch):
                """K^T[:, tch] and V_aug[tch] from x^T chunk tch."""
                xt = xts[tch]
                # K: accumulate over cs for 4-pair groups
                for g in range(2):
                    pk = pmm.tile([P, 4, QCH], F32, tag="mm", name="pk")
                    for pp in range(4):
                        p = g * 4 + pp
                        for cs in range(CSUB):
                            nc.tensor.matmul(
                                pk[:, pp, :],
                                wk[:, cs, p * P:(p + 1) * P],
                                xt[:, cs, :],
                                start=(cs == 0), stop=(cs == CSUB - 1),
                            )
                    nc.scalar.copy(
                        out=kts[tch][:, g * 4:(g + 1) * 4, :], in_=pk)
                # V: two 128-row subtiles per chunk, two 512-col halves
                for ts in range(2):
                    for och in range(2):
                        pv = pmm.tile([P, 4, QCH], F32, tag="mm", name="pv")
                        pvv = pv.rearrange("a b c -> a (b c)")[:, 0:512]
                        for cs in range(CSUB):
                            nc.tensor.matmul(
                                pvv,
                                xt[:, cs, ts * P:(ts + 1) * P],
                                wv[:, cs, och * 512:(och + 1) * 512],
                                start=(cs == 0), stop=(cs == CSUB - 1),
                            )
                        # pv cols = (pair-in-half 4, hi 2, d 64)
                        pvh = pvv.rearrange("a (b s d) -> a b s d", b=4, s=2)
                        p4 = slice(och * 4, (och + 1) * 4)
                        nc.vector.tensor_copy(
                            out=vaugs[tch][:, ts, p4, 0:64],
                            in_=pvh[:, :, 0, :])
                        nc.vector.tensor_copy(
                            out=vaugs[tch][:, ts, p4, 65:129],
                            in_=pvh[:, :, 1, :])

            def stage_b(s):
                """Q^T for slot s."""
                xq = xqs[s]
                for g in range(2):
                    pq = pmm.tile([P, 4, QCH], F32, tag="mm", name="pq")
                    for pp in range(4):
                        p = g * 4 + pp
                        for cs in range(CSUB):
                            nc.tensor.matmul(
                                pq[:, pp, :],
                                wq[:, cs, p * P:(p + 1) * P],
                                xq[:, cs, :],
                                start=(cs == 0), stop=(cs == CSUB - 1),
                            )
                    nc.vector.tensor_copy(
                        out=qts[s][:, g * 4:(g + 1) * 4, :], in_=pq)

            def stage_c(s, ysT, filler=None):
                """Attention for slot s into resident bf16 y^T tile.
                filler: list of thunks (e.g. prev slot's D groups) emitted
                between pairs to fill PE bubbles left by the exp chain."""
                nj = BOUNDS[s]
                ngrp = nj // 4

                def norm(p, ypp):
                    # batched 1/l for both heads, rank-1 PE broadcast, then
                    # one SBUF copy of the broadcast + two scale-muls
                    rinv = nrm.tile([P, 2, QCH], BF16, tag="rinv", name="rinv")
                    with nc.allow_low_precision(reason="1/l in bf16"):
                        nc.vector.reciprocal(
                            out=rinv[64:65, :, :], in_=ypp[64:65, :, :])
                    lb = pmm.tile([P, 4, QCH], F32, tag="mm", name="lb")
                    nc.tensor.matmul(
                        lb[0:64, 0:2, :], onesb[64:65, :], rinv[64:65, :, :],
                        start=True, stop=True,
                    )
                    lbs = nrm.tile([64, 2, QCH], BF16, tag="lbs", name="lbs")
                    nc.vector.tensor_copy(out=lbs, in_=lb[0:64, 0:2, :])
                    nc.vector.tensor_mul(
                        ysT[0:64, p, :], ypp[0:64, 0, :], lbs[:, 0, :])
                    # hi1 lives at ysT partitions 64..127: go through a
                    # base-0 staging tile + lane-crossing SBUF->SBUF DMA
                    ysb = nrm.tile([64, QCH], BF16, tag="ysb", name="ysb")
                    nc.vector.tensor_mul(
                        ysb, ypp[0:64, 1, :], lbs[:, 1, :])
                    nc.sync.dma_start(out=ysT[64:128, p, :], in_=ysb)

                ypps = [None] * NPAIR
                for p in range(NPAIR):
                    ypp = ppy.tile([P, 2, QCH], F32, tag="ypp", name="ypp")
                    ypps[p] = ypp
                    for hi in range(2):
                        h0 = hi * 64
                        # both heads: rows 0..63 y, row 64 l
                        yout = ypp[0:65, hi, :]
                        vsl = slice(hi * 65, hi * 65 + 65)  # [V | one]
                        for g in range(ngrp):
                            st4 = pmm.tile([P, 4, QCH], F32, tag="mm", name="st4")
                            for i in range(4):
                                jt = g * 4 + i
                                nc.tensor.matmul(
                                    st4[:, i, :],
                                    kts[jt // 2][h0:h0 + 64, p,
                                                 (jt % 2) * P:(jt % 2 + 1) * P],
                                    qts[s][h0:h0 + 64, p, :],
                                    start=True, stop=True,
                                )
                            pt4 = wrk.tile([P, 4, QCH], BF16, tag="pt", name="pt4")
                            nc.scalar.activation(
                                out=pt4, in_=st4,
                                func=mybir.ActivationFunctionType.Exp,
                                scale=SCALE,
                            )
                            if g == ngrp - 1:
                                nc.vector.tensor_mul(pt4, pt4, mask[:, s, :, :])
                            for i in range(4):
                                jt = g * 4 + i
                                nc.tensor.matmul(
                                    yout,
                                    vaugs[jt // 2][:, jt % 2, p, vsl],
                                    pt4[:, i, :],
                                    start=(jt == 0), stop=(jt == nj - 1),
                                )
                    # normalization of the previous pair (deferred so the
                    # norm's PE broadcast never stalls this pair's matmuls)
                    if p >= 1:
                        norm(p - 1, ypps[p - 1])
                    if filler and p % 2 == 1:
                        filler[p // 2]()
                norm(NPAIR - 1, ypps[NPAIR - 1])

            def stage_d_groups(s, ysT):
                """Output projection for slot s as 4 deferred thunks."""
                def mk(qh, och):
                    def emit():
                        po = pmm.tile([P, 4, QCH], F32, tag="mm", name="po")
                        pov = po.rearrange("a b c -> a (b c)")[:, 0:512]
                        for cb in range(CSUB):
                            nc.tensor.matmul(
                                pov,
                                ysT[:, cb, qh * P:(qh + 1) * P],
                                wp[:, cb, och * 512:(och + 1) * 512],
                                start=(cb == 0), stop=(cb == CSUB - 1),
                            )
                        osb = wrk.tile([P, 512], F32, tag="osb", name="osb")
                        nc.vector.tensor_copy(out=osb, in_=pov)
                        nc.sync.dma_start(
                            out=out[s * QCH + qh * P:s * QCH + (qh + 1) * P,
                                    och * 512:(och + 1) * 512],
                            in_=osb,
                        )
                    return emit
                return [mk(qh, och) for qh in range(2) for och in range(2)]

            # ---- the pipeline ----
            prev_d = None
            for s in range(NSLOT):
                for half in range(2):
                    tch = 2 * s + half
                    if xts[tch] is None:
                        xts[tch] = xp.tile(
                            [P, CSUB, QCH], BF16, tag="xt", name=f"xt{tch}")
                        nc.sync.dma_start(
                            out=xts[tch],
                            in_=xTv[:, :, tch * QCH:(tch + 1) * QCH])
                    stage_a(tch)
                if xqs[s] is None:
                    xqs[s] = xqp.tile(
                        [P, CSUB, QCH], BF16, tag="xq", name=f"xq{s}")
                    nc.sync.dma_start(
                        out=xqs[s],
                        in_=xTqv[:, :, s * QCH:(s + 1) * QCH])
                stage_b(s)
                ysT = ysp.tile([P, NPAIR, QCH], BF16, tag="ys", name=f"ys{s}")
                stage_c(s, ysT, filler=prev_d)
                prev_d = stage_d_groups(s, ysT)
            for t in prev_d:
                t()

    nc.compile()
    return nc


def _make_mask(parity: int) -> np.ndarray:
    import ml_dtypes
    m = np.zeros((P, NSLOT, 4, QCH), dtype=np.float32)
    for s in range(NSLOT):
        c = CHUNKS[parity][s]
        for i in range(4):
            jt = BOUNDS[s] - 4 + i
            jg = jt * P + np.arange(P)[:, None]          # key index
            qg = c * QCH + np.arange(QCH)[None, :]       # query index
            m[:, s, i, :] = np.where(jg <= qg, 1.0, 0.0)
    return m.astype(ml_dtypes.bfloat16)


def kernel(x, Wq, bq, Wk, bk, Wv, bv, Wp, bp):
    import ml_dtypes
    x = np.asarray(x, dtype=np.float32)
    assert x.shape == (B, T, C)
    for b_ in (bq, bk, bv, bp):
        assert not np.any(np.asarray(b_)), "nonzero biases unsupported"

    if "nc" not in _CACHE:
        _CACHE["nc"] = build_nc()
    nc = _CACHE["nc"]

    bf = ml_dtypes.bfloat16
    wqT = np.ascontiguousarray(np.asarray(Wq, np.float32).T).astype(bf)
    wkT = np.ascontiguousarray(np.asarray(Wk, np.float32).T).astype(bf)
    wvT = np.ascontiguousarray(np.asarray(Wv, np.float32).T).astype(bf)
    wpT = np.ascontiguousarray(np.asarray(Wp, np.float32).T).astype(bf)
    masks = [_make_mask(0), _make_mask(1)]

    in_maps = []
    for core in range(8):
        b, par = core // 2, core % 2
        xTf = np.ascontiguousarray(x[b].T)
        xT = xTf.astype(bf)
        xTq = np.ascontiguousarray(
            np.concatenate(
                [xTf[:, c * QCH:(c + 1) * QCH] for c in CHUNKS[par]], axis=1)
        ).astype(bf)
        in_maps.append(
            dict(xT=xT, xTq=xTq, wqT=wqT, wkT=wkT, wvT=wvT, wpT=wpT,
                 mask=masks[par])
        )

    _CACHE["last_in_maps"] = in_maps
    try:
        res = run_bass_kernel_spmd(nc, in_maps, core_ids=list(range(8)))
    except Exception:
        # the axon device occasionally reports NRT_EXEC_UNIT_UNRECOVERABLE;
        # resetting the PJRT backend and retrying once recovers it
        import jax
        try:
            jax.clear_caches()
            jax.extend.backend.clear_backends()
        except Exception:
            pass
        res = run_bass_kernel_spmd(nc, in_maps, core_ids=list(range(8)))

    out = np.empty((B, T, C), dtype=np.float32)
    for core in range(8):
        ol = res.results[core]["out"]
        b, par = core // 2, core % 2
        for s, c in enumerate(CHUNKS[par]):
            out[b, c * QCH:(c + 1) * QCH] = ol[s * QCH:(s + 1) * QCH]
    return out
